# revision 83
# baseline (speedup 1.0000x reference)
"""3D window attention (B=32, N=513, D=768, H=12) on 8 trn2 NeuronCores. v6

Data-parallel over batch (4 per core). bf16 datapath:
  host: x pre-transposed to xT [128, 6ci, T] (no on-device transpose phase);
        relative-position bias gathered and packed to fp8 x8-prescaled
        tables for PE DoubleRow injection; exact f32 exp(bias) for tails.
  B2:   q[g] = [128(2 heads), T] bf16, k[g] = [128, T] bf16, in SBUF;
        batch-outer so each per-batch xT DMA chunk unblocks 12 psum tiles.
  B:    v[b] = [128, 4 key-tiles + key-512 row, 12*(64+1)] bf16 with ones
        column (denominator ride-along); key-512 v rows at partition 0/64.
  C:    per (b,g): S = fp8-DR bias inject + K^T Q, exp(S*0.125) on ACT,
        AV [65,512] accumulation. Key 512 is a rank-1 path (partitions 0/64
        of one bank, hoisted to the iteration start). The tail query's ops
        are deferred one iteration; the U-normalize (recip -> partition
        broadcast -> mul) is queued first on DVE after the AV stop so the
        two psU banks recycle with minimum latency.
  D:    y = a @ Wp^T + b_proj, interleaved 2 passes/iteration into C
        (emitted before the normalize so its DVE add isn't queue-blocked),
        drained with psS-bank alternation at the end.
"""

import os
import numpy as np

PROBE = set(os.environ.get("KPROBE", "").split(","))

B, N_TOK, DIM, HEADS = 32, 513, 768, 12
HD = DIM // HEADS             # 64
N_CORES = 8
B_PER = B // N_CORES          # 4
T = B_PER * N_TOK             # 2052
G = HEADS // 2                # 6 head pairs
ES = 0.125                    # exp scale (= head_dim**-0.5)

_CACHE = {}


def _build_nc():
    import concourse.bacc as bacc
    import concourse.mybir as mybir
    import concourse.tile as tile
    from concourse.ap import AP

    F32 = mybir.dt.float32
    BF16 = mybir.dt.bfloat16
    FP8 = mybir.dt.float8e4
    AF = mybir.ActivationFunctionType
    DR = mybir.MatmulPerfMode.DoubleRow

    nc = bacc.Bacc("TRN2", target_bir_lowering=False, debug=False)

    xT_d = nc.dram_tensor("xT", [128, 6 * T], BF16, kind="ExternalInput")
    wqk_d = nc.dram_tensor("wqkT", [DIM, 2 * DIM], BF16, kind="ExternalInput")
    wv_d = nc.dram_tensor("wvT", [DIM, DIM], BF16, kind="ExternalInput")
    bt8_d = nc.dram_tensor("bt8", [G, 128, 8 * 512], FP8, kind="ExternalInput")
    bt4_d = nc.dram_tensor("bt4", [2, G * 512], FP8, kind="ExternalInput")
    id0_d = nc.dram_tensor("id0", [128, 256], FP8, kind="ExternalInput")
    e2_d = nc.dram_tensor("e2", [2, 256], FP8, kind="ExternalInput")
    ebt_d = nc.dram_tensor("ebt", [128, 10 * G], F32, kind="ExternalInput")
    wp_d = nc.dram_tensor("wpT", [DIM, DIM], BF16, kind="ExternalInput")
    bb_d = nc.dram_tensor("bb", [128, DIM], F32, kind="ExternalInput")
    y_d = nc.dram_tensor("y", [T, DIM], F32, kind="ExternalOutput")

    t_tiles = [(i * 128, min(128, T - i * 128)) for i in range((T + 127) // 128)]

    with tile.TileContext(nc) as tc:
        consts_cm = tc.tile_pool(name="consts", bufs=1)
        consts = consts_cm.__enter__()
        pers_cm = tc.tile_pool(name="pers", bufs=1)
        pers = pers_cm.__enter__()
        xTp_cm = tc.tile_pool(name="xTp", bufs=1)
        xTp = xTp_cm.__enter__()

        bb = consts.tile([128, DIM], F32, tag="bb", name="bb")
        ebt = consts.tile([128, 10 * G], F32, tag="ebt", name="ebt")
        id0 = consts.tile([128, 256], FP8, tag="id0", name="id0")
        e2 = consts.tile([2, 256], FP8, tag="e2", name="e2")
        bt4S = consts.tile([2, G * 512], FP8, tag="bt4", name="bt4")

        qS = [pers.tile([128, T], BF16, tag=f"q{g}", name=f"q{g}")
              for g in range(G)]
        kS = [pers.tile([128, T], BF16, tag=f"k{g}", name=f"k{g}")
              for g in range(G)]
        btS = [pers.tile([128, 8 * 512], FP8, tag=f"bt{g}", name=f"bt{g}")
               for g in range(G)]
        v8 = [pers.tile([128, 5, HEADS * (HD + 1)], BF16, tag=f"v{b}",
                        name=f"v{b}") for b in range(B_PER)]
        wvp_cm = tc.tile_pool(name="wvp", bufs=1)
        wvp = wvp_cm.__enter__()
        wqp_cm = tc.tile_pool(name="wqp", bufs=1)
        wqp = wqp_cm.__enter__()
        wqk = wqp.tile([128, 6, 2 * DIM], BF16, tag="wqk", name="wqk")
        wv = wvp.tile([128, 6, DIM], BF16, tag="wv", name="wv")
        xT = xTp.tile([128, 6, T], BF16, tag="xT", name="xT")

        wqk_src = wqk_d.ap().rearrange("(c p) d -> p c d", p=128)
        wv_src = wv_d.ap().rearrange("(c p) d -> p c d", p=128)
        xT_src = xT_d.ap().rearrange("p (c t) -> p c t", t=T)

        # input DMAs, ordered so B2 unblocks earliest: wqk ci-chunks with
        # xT batch-chunks woven in (B2 consumes per-batch), then wv, bias.
        xT_bsrc = xT_d.ap().rearrange("p (c b n) -> p c b n", c=6, n=N_TOK)
        xT_bdst = xT.rearrange("p c (b n) -> p c b n", n=N_TOK)
        # fine-grained prelude: q-half of wqk-ci0, then xT ci-chunks of
        # batch 0 woven between wqk ci-chunks, so the first projection
        # matmuls start ~2.5us in and the PE p-state ramps without resets
        nc.sync.dma_start(wqk[:, 0, 0:DIM], wqk_src[:, 0, 0:DIM])
        nc.sync.dma_start(xT_bdst[:, 0, 0], xT_bsrc[:, 0, 0])
        nc.sync.dma_start(wqk[:, 0, DIM:2 * DIM], wqk_src[:, 0, DIM:2 * DIM])
        nc.sync.dma_start(xT_bdst[:, 1, 0], xT_bsrc[:, 1, 0])
        nc.sync.dma_start(wqk[:, 1], wqk_src[:, 1])
        for ci in range(2, 6):
            nc.sync.dma_start(xT_bdst[:, ci, 0], xT_bsrc[:, ci, 0])
            nc.sync.dma_start(wqk[:, ci], wqk_src[:, ci])
        for b in range(1, B_PER):
            nc.sync.dma_start(xT_bdst[:, :, b], xT_bsrc[:, :, b])
        for ci in range(6):
            nc.sync.dma_start(wv[:, ci], wv_src[:, ci])
        for g in range(G):
            nc.sync.dma_start(btS[g][:], bt8_d.ap()[g])
        nc.sync.dma_start(bb[:], bb_d.ap())
        nc.sync.dma_start(ebt[:], ebt_d.ap())
        nc.sync.dma_start(id0[:], id0_d.ap())
        nc.sync.dma_start(e2[:], e2_d.ap())
        nc.sync.dma_start(bt4S[:], bt4_d.ap())
        id0p = id0.rearrange("p (i c) -> p i c", c=128)
        e2p = e2.rearrange("p (i c) -> p i c", c=128)

        for b in range(B_PER):
            v3 = v8[b].rearrange("p m (h x) -> p m h x", x=HD + 1)
            nc.gpsimd.memset(v3[:, :, :, HD], 1.0)

        # ---------------- Phase B2: q / k in SBUF ----------------
        pB2_ps = tc.tile_pool(name="psB2", bufs=7, space="PSUM")
        pB2_pt = tc.tile_pool(name="psB2t", bufs=1, space="PSUM")
        with pB2_ps as psB2, pB2_pt as psB2t:
            cast_i = 0

            def cast(out, in_):
                nonlocal cast_i
                cast_i += 1
                if cast_i % 2 == 0:
                    nc.vector.tensor_copy(out, in_)
                else:
                    nc.scalar.copy(out, in_)

            # batch-outer so each xT batch-chunk DMA unblocks 12 full
            # psum tiles of projection work (q and k, all 6 groups)
            for b in range(B_PER):
                for g in range(G):
                    psq = psB2.tile([128, 512], F32, tag="psq", name="psq")
                    for ci in range(6):
                        nc.tensor.matmul(
                            psq[:], wqk[:, ci, g * 128:(g + 1) * 128],
                            xT[:, ci, b * N_TOK:b * N_TOK + 512],
                            start=(ci == 0), stop=(ci == 5))
                    cast(qS[g][:, b * N_TOK:b * N_TOK + 512], psq[:])
                    psk = psB2.tile([128, 512], F32, tag="psq", name="psk")
                    for ci in range(6):
                        nc.tensor.matmul(
                            psk[:], wqk[:, ci, DIM + g * 128:DIM + (g + 1) * 128],
                            xT[:, ci, b * N_TOK:b * N_TOK + 512],
                            start=(ci == 0), stop=(ci == 5))
                    cast(kS[g][:, b * N_TOK:b * N_TOK + 512], psk[:])
            for g in range(G):
                psqt = psB2t.tile([128, 4], F32, tag="psqt", name="psqt")
                for ci in range(6):
                    nc.tensor.matmul(
                        psqt[:], wqk[:, ci, g * 128:(g + 1) * 128],
                        xT.rearrange("p c (b n) -> p c b n", n=N_TOK)
                          [:, ci, :, 512],
                        start=(ci == 0), stop=(ci == 5))
                cast(qS[g][:, 0:T].rearrange(
                         "p (b w) -> p b w", w=N_TOK)[:, :, 512],
                     psqt[:])
                psk4 = psB2t.tile([128, 4], F32, tag="psqt", name="psk4")
                for ci in range(6):
                    nc.tensor.matmul(
                        psk4[:], wqk[:, ci, DIM + g * 128:DIM + (g + 1) * 128],
                        xT.rearrange("p c (b n) -> p c b n", n=N_TOK)
                          [:, ci, :, 512],
                        start=(ci == 0), stop=(ci == 5))
                cast(kS[g][:, 0:T].rearrange(
                         "p (b w) -> p b w", w=N_TOK)[:, :, 512],
                     psk4[:])

        wqp_cm.__exit__(None, None, None)

        # ---------------- Phase B: v (per-batch key-tile layout) ---------
        pB_ps = tc.tile_pool(name="psB", bufs=3, space="PSUM")
        pB_pt = tc.tile_pool(name="psBt", bufs=1, space="PSUM")
        with pB_ps as psB, pB_pt as psBt:
            for b in range(B_PER):
                v3 = v8[b].rearrange("p m (h x) -> p m h x", x=HD + 1)
                for mt in range(4):
                    psv = psB.tile([128, DIM], F32, tag="psv", name="psv")
                    lhs0 = b * N_TOK + mt * 128
                    for ci in range(6):
                        lhsT = xT[:, ci, lhs0:lhs0 + 128]
                        nc.tensor.matmul(psv[:, 0:512], lhsT,
                                         wv[:, ci, 0:512],
                                         start=(ci == 0), stop=(ci == 5))
                        nc.tensor.matmul(psv[:, 512:768], lhsT,
                                         wv[:, ci, 512:768],
                                         start=(ci == 0), stop=(ci == 5))
                    src = psv[:].rearrange("p (h d) -> p h d", d=HD)
                    if b % 2 == 0:
                        nc.vector.tensor_copy(v3[:, mt, :, 0:HD], src)
                    else:
                        nc.scalar.copy(v3[:, mt, :, 0:HD], src)
            psvt = psBt.tile([128, DIM], F32, tag="psvt", name="psvt")
            for ci in range(6):
                lhsT = xT.rearrange("p c (b n) -> p c b n", n=N_TOK)[:, ci, :, 512]
                nc.tensor.matmul(psvt[:B_PER, 0:512], lhsT, wv[:, ci, 0:512],
                                 start=(ci == 0), stop=(ci == 5))
                nc.tensor.matmul(psvt[:B_PER, 512:768], lhsT, wv[:, ci, 512:768],
                                 start=(ci == 0), stop=(ci == 5))
            vst = pers.tile([B_PER, DIM], BF16, tag="vst", name="vst")
            nc.vector.tensor_copy(vst[:], psvt[:B_PER, :])
            for b in range(B_PER):
                v3 = v8[b].rearrange("p m (h x) -> p m h x", x=HD + 1)
                vsrc = vst.rearrange("p (h d) -> p h d", d=HD)
                # key-512 v rows: even heads (h2=0) at partition 0, odd heads
                # (h2=1) at partition 64, matching the pmb4/ptail4 row homes
                nc.sync.dma_start(v3[0:1, 4, 0:12:2, 0:HD],
                                  vsrc[b:b + 1, 0:12:2])
                nc.sync.dma_start(v3[HD:HD + 1, 4, 1:12:2, 0:HD],
                                  vsrc[b:b + 1, 1:12:2])

        wvp_cm.__exit__(None, None, None)
        xTp_cm.__exit__(None, None, None)

        # ---------------- Phase C + interleaved D ----------------
        aTp_cm = tc.tile_pool(name="aTp", bufs=1)
        aTp = aTp_cm.__enter__()
        aT = [aTp.tile([128, T], BF16, tag=f"aT{g}", name=f"aT{g}")
              for g in range(G)]
        wpal = aTp.tile([128, 6, DIM], BF16, tag="wpal", name="wpal")
        wp_src = wp_d.ap().rearrange("(c p) d -> p c d", p=128)
        for ci in range(6):
            nc.sync.dma_start(wpal[:, ci], wp_src[:, ci])
        pC_pm = tc.tile_pool(name="pmp", bufs=4)
        pC_pt = tc.tile_pool(name="ptp", bufs=2)
        pC_sm = tc.tile_pool(name="smallp", bufs=3)
        pD3 = tc.tile_pool(name="ytp", bufs=3)
        pS_ps = tc.tile_pool(name="psS", bufs=2, space="PSUM")
        pU_ps = tc.tile_pool(name="psU", bufs=2, space="PSUM")
        pT_ps = tc.tile_pool(name="psT", bufs=1, space="PSUM")
        pD_ps = tc.tile_pool(name="psD", bufs=1, space="PSUM")
        d_done = [0]
        d_half = [0]
        d_yt = [None]
        d_avail = [0]
        with pC_pm as pmp, \
             pC_pt as ptp, pC_sm as smallp, pD3 as ytp, \
             pS_ps as psS, pU_ps as psU, pT_ps as psT, pD_ps as psD:

            def emit_d_pass(alt=False):
                # one 6-matmul pass (half an output tile) per call; the
                # half-0 call sits right before the normalize block so its
                # yt-add beats the norm muls onto the DVE queue.
                # alt=True (drain) alternates banks via the idle psS pool.
                if d_done[0] >= len(t_tiles):
                    return False
                t0, ts = t_tiles[d_done[0]]
                if t0 + ts > d_avail[0]:
                    return False
                half = d_half[0]
                (c0, cw) = ((0, 512), (512, 256))[half]
                if alt and half == 1:
                    psy = psS.tile([128, 1024], F32, tag="S",
                                   name="psyS")[:, 0:512]
                else:
                    psy = psD.tile([128, 512], F32, tag="psy", name="psy")
                for j in range(6):
                    nc.tensor.matmul(psy[:ts, 0:cw], aT[j][:, t0:t0 + ts],
                                     wpal[:, j, c0:c0 + cw],
                                     start=(j == 0), stop=(j == 5))
                if half == 0:
                    yt = ytp.tile([128, DIM], F32, tag="yt", name="yt")
                    d_yt[0] = yt
                    nc.vector.tensor_add(yt[:ts, 0:512],
                                         psy[:ts, 0:512], bb[:ts, 0:512])
                    d_half[0] = 1
                else:
                    yt = d_yt[0]
                    nc.vector.tensor_add(yt[:ts, 512:768],
                                         psy[:ts, 0:256], bb[:ts, 512:768])
                    nc.sync.dma_start(y_d.ap()[t0:t0 + ts, :], yt[:ts, :])
                    d_half[0] = 0
                    d_done[0] += 1
                return True

            def emit_tail_av(st):
                # AV for the tail query of iteration st (ptail ready by now)
                b, g = st["b"], st["g"]
                for h2 in range(2):
                    h = 2 * g + h2
                    for mt in range(4):
                        nc.tensor.matmul(
                            st["stail"][0:HD + 1, 10 + h2:11 + h2],
                            v8[b][:, mt, h * 65:h * 65 + 65],
                            st["ptail"][:, 2 * mt + h2:2 * mt + h2 + 1],
                            start=(mt == 0), stop=False)
                    nc.tensor.matmul(
                        st["stail"][0:HD + 1, 10 + h2:11 + h2],
                        v8[b][h2 * HD:h2 * HD + 1, 4, h * 65:h * 65 + 65],
                        st["ptail"][h2 * HD:h2 * HD + 1, 8 + h2:9 + h2],
                        start=False, stop=True)

            def emit_norm_tail(st):
                b, g = st["b"], st["g"]
                rnt = smallp.tile([1, 2], F32, tag="rnt", name="rnt")
                nc.vector.reciprocal(rnt[:], st["stail"][HD:HD + 1, 10:12])
                for h2 in range(2):
                    bct = smallp.tile([HD, 1], F32, tag="bct", name="bct")
                    nc.gpsimd.partition_broadcast(bct[:], rnt[:, h2:h2 + 1])
                    nc.vector.tensor_mul(
                        aT[g][h2 * HD:h2 * HD + HD,
                              b * N_TOK + 512:b * N_TOK + 513],
                        st["stail"][0:HD, 10 + h2:11 + h2], bct[:])

            prev = [None]

            for b in range(B_PER):
                for g in range(G):
                    q, k = qS[g], kS[g]
                    q0 = b * N_TOK
                    btp = btS[g][:]
                    bt_pitch = btp.ap[0][0]

                    def s_main(mt):
                        # fp8 DoubleRow bias inject (half PE cost), then K^T Q
                        sm = psS.tile([128, 1024], F32, tag="S", name="S")
                        for h2 in range(2):
                            nc.tensor.matmul(
                                sm[:, h2 * 512:h2 * 512 + 512], id0p[:],
                                AP(btp.tensor, (h2 * 4 + mt) * 512,
                                   [[bt_pitch, 128], [0, 2], [1, 512]]),
                                start=True, stop=False, perf_mode=DR)
                            nc.tensor.matmul(
                                sm[:, h2 * 512:h2 * 512 + 512],
                                k[h2 * HD:h2 * HD + HD,
                                  q0 + mt * 128:q0 + mt * 128 + 128],
                                q[h2 * HD:h2 * HD + HD, q0:q0 + 512],
                                start=False, stop=True)
                        return sm

                    def exp_main(sm):
                        pm = pmp.tile([128, 1024], BF16, tag="pm", name="pm")
                        nc.scalar.activation(pm[:], sm[:], AF.Exp, bias=0.0,
                                             scale=ES)
                        return pm

                    def emit_av(mt, pmb, stop=False):
                        for h2 in range(2):
                            h = 2 * g + h2
                            nc.tensor.matmul(
                                U2[h2][:],
                                v8[b][:, mt, h * 65:h * 65 + 65],
                                pmb[:, h2 * 512:h2 * 512 + 512],
                                start=(mt == 0), stop=stop)

                    if g % 2 == 0:
                        emit_d_pass()
                    U2 = [psU.tile([HD + 1, 512], F32, tag="U", name="U")
                          for _ in range(2)]
                    # key-512 rank-1 row first: exp4 fills the ACT engine's
                    # iteration-boundary idle instead of delaying exp2/exp3
                    if "noexp4" not in PROBE:
                        sm4 = psS.tile([128, 1024], F32, tag="S", name="S4")
                        bt4p = bt4S[:]
                        nc.tensor.matmul(
                            sm4[:, 0:512], e2p[:],
                            AP(bt4p.tensor, g * 512,
                               [[bt4p.ap[0][0], 2], [0, 2], [1, 512]]),
                            start=True, stop=True, perf_mode=DR)
                        for h2 in range(2):
                            nc.tensor.matmul(
                                sm4[h2 * HD:h2 * HD + 1, 0:512],
                                k[h2 * HD:h2 * HD + HD, q0 + 512:q0 + 513],
                                q[h2 * HD:h2 * HD + HD, q0:q0 + 512],
                                start=False, stop=True)
                        pm4 = pmp.tile([128, 1024], BF16, tag="pm", name="pm4")
                        nc.scalar.activation(pm4[:, 0:512], sm4[:, 0:512],
                                             AF.Exp, bias=0.0, scale=ES)
                    sm0 = s_main(0)
                    pm0 = exp_main(sm0)
                    sm1 = s_main(1)
                    if prev[0] is not None:
                        emit_tail_av(prev[0])
                        if g == 0 and b > 0:
                            d_avail[0] = b * N_TOK
                    pm1 = exp_main(sm1)
                    sm2 = s_main(2)
                    if prev[0] is not None:
                        emit_norm_tail(prev[0])
                    pm2 = exp_main(sm2)
                    sm3 = s_main(3)
                    emit_av(0, pm0)
                    emit_av(1, pm1)
                    # key-512 rank-1 AV early (AV3 is the group stop)
                    if "noexp4" not in PROBE:
                        for h2 in range(2):
                            h = 2 * g + h2
                            nc.tensor.matmul(
                                U2[h2][:],
                                v8[b][h2 * HD:h2 * HD + 1, 4,
                                      h * 65:h * 65 + 65],
                                pm4[h2 * HD:h2 * HD + 1, 0:512],
                                start=False, stop=False)
                    pm3 = exp_main(sm3)
                    # tail-query logits (consumed next iteration)
                    if "notailq" not in PROBE:
                        stail = psT.tile([128, 12], F32, tag="st", name="st")
                        for h2 in range(2):
                            for mt in range(4):
                                nc.tensor.matmul(
                                    stail[:, 2 * mt + h2:2 * mt + h2 + 1],
                                    k[h2 * HD:h2 * HD + HD,
                                      q0 + mt * 128:q0 + mt * 128 + 128],
                                    q[h2 * HD:h2 * HD + HD, q0 + 512:q0 + 513],
                                    start=True, stop=True)
                            nc.tensor.matmul(
                                stail[h2 * HD:h2 * HD + 1, 8 + h2:9 + h2],
                                k[h2 * HD:h2 * HD + HD, q0 + 512:q0 + 513],
                                q[h2 * HD:h2 * HD + HD, q0 + 512:q0 + 513],
                                start=True, stop=True)
                        praw = smallp.tile([128, 10], F32, tag="praw",
                                           name="praw")
                        nc.scalar.activation(praw[:], stail[:, 0:10], AF.Exp,
                                             bias=0.0, scale=ES)
                    emit_av(2, pm2)
                    emit_av(3, pm3, stop=True)
                    emit_d_pass()
                    # normalize main block FIRST on the DVE queue after the
                    # U-group stop, so the psU buffers free with minimum
                    # latency (next iteration's AV(0) reuses them).
                    rns = []
                    for h2 in range(2):
                        if "norecip" in PROBE:
                            continue
                        rn = smallp.tile([1, 512], F32, tag="rn", name="rn")
                        nc.vector.reciprocal(rn[:], U2[h2][HD:HD + 1, :])
                        rns.append(rn)
                    for h2 in range(2):
                        if "norecip" in PROBE:
                            nc.vector.tensor_copy(
                                aT[g][h2 * HD:h2 * HD + HD, q0:q0 + 512],
                                U2[h2][0:HD, :])
                            continue
                        bc = smallp.tile([HD, 512], F32, tag="bc", name="bc")
                        nc.gpsimd.partition_broadcast(bc[:], rns[h2][:])
                        nc.vector.tensor_mul(
                            aT[g][h2 * HD:h2 * HD + HD, q0:q0 + 512],
                            U2[h2][0:HD, :], bc[:])
                    if "notailq" not in PROBE:
                        ptail = ptp.tile([128, 10], BF16, tag="pt", name="pt")
                        nc.vector.tensor_mul(ptail[:], praw[:],
                                             ebt[:, g * 10:g * 10 + 10])
                        prev[0] = {"b": b, "g": g, "stail": stail,
                                   "ptail": ptail}
                    elif g == 0 and b > 0:
                        d_avail[0] = b * N_TOK

            if prev[0] is not None:
                emit_tail_av(prev[0])
                emit_norm_tail(prev[0])
            d_avail[0] = T
            while emit_d_pass(alt=True):
                pass

        aTp_cm.__exit__(None, None, None)
        pers_cm.__exit__(None, None, None)
        consts_cm.__exit__(None, None, None)

    nc.compile()
    return nc


def get_nc():
    if "nc" not in _CACHE:
        _CACHE["nc"] = _build_nc()
    return _CACHE["nc"]


def host_prep(w_qkv, bias_table, w_proj, b_proj, rel_index):
    """Host-side packing shared by all cores."""
    import ml_dtypes
    BF = ml_dtypes.bfloat16
    w = np.asarray(w_qkv, dtype=np.float32)
    wqkT = np.ascontiguousarray(w[0:2 * DIM].T).astype(BF)     # [c, 1536]
    wvT = np.ascontiguousarray(w[2 * DIM:3 * DIM].T).astype(BF)

    E4 = ml_dtypes.float8_e4m3
    BPK = 8.0                                   # bias prepack = 1/ES
    tbl = np.asarray(bias_table, dtype=np.float32)
    gat = tbl[np.asarray(rel_index)]            # [n(query), m(key), h]
    Bm = gat.transpose(2, 1, 0)                 # [h, m(key), n(query)]
    EBm = np.exp(Bm)                            # exp(bias) for the tail path

    # mt 2/3: fp8 pre-scaled (x8) bias tables, injected into S on the PE;
    # mt 0/1: exact bf16 exp(bias) factors, applied on the DVE after exp
    bt8 = np.zeros((G, 128, 8, 512), dtype=np.float32)
    for g in range(G):
        for h2 in range(2):
            for mt in range(4):
                bt8[g, :, h2 * 4 + mt, :] = \
                    BPK * Bm[2 * g + h2, mt * 128:mt * 128 + 128, 0:512]
    bt8 = np.clip(bt8, -240, 240).reshape(G, 128, 4096).astype(E4)

    # key-512 row biases: bt4 rows 0 / 2 pair with e2 one-hots at cols 0 / 64
    bt4 = np.zeros((2, G * 512), dtype=np.float32)
    for g in range(G):
        for h2 in range(2):
            bt4[h2, g * 512:g * 512 + 512] = \
                BPK * Bm[2 * g + h2, 512, 0:512]
    bt4 = np.clip(bt4, -240, 240).astype(E4)

    id0 = np.zeros((128, 256), dtype=np.float32)
    id0[:, 0:128] = np.eye(128, dtype=np.float32)
    id0 = id0.astype(E4)
    e2 = np.zeros((2, 256), dtype=np.float32)
    e2[0, 0] = 1.0      # bt4 row 0 -> sm4 partition 0   (h2=0)
    e2[1, HD] = 1.0     # bt4 row 1 -> sm4 partition 64  (h2=1)
    e2 = e2.astype(E4)

    ebt = np.zeros((128, 10 * G), dtype=np.float32)
    for g in range(G):
        for mt in range(4):
            for h2 in range(2):
                ebt[:, g * 10 + 2 * mt + h2] = \
                    EBm[2 * g + h2, mt * 128:mt * 128 + 128, 512]
        for h2 in range(2):
            # cols 8/9: key-512 factor at row h2*HD (the partition home of
            # the rank-1 path); other rows stay 0 and mask stale exp lanes
            ebt[h2 * HD, g * 10 + 8 + h2] = EBm[2 * g + h2, 512, 512]

    wpT = np.ascontiguousarray(np.asarray(w_proj, dtype=np.float32).T).astype(BF)
    bb = np.ascontiguousarray(
        np.broadcast_to(np.asarray(b_proj, dtype=np.float32), (128, DIM)))
    return {"wqkT": wqkT, "wvT": wvT, "bt8": bt8, "bt4": bt4,
            "id0": id0, "e2": e2, "ebt": ebt, "wpT": wpT, "bb": bb}


def prep_x(x_core):
    """[B_PER, N_TOK, DIM] f32 -> xT [128, 6*T] bf16 (host transpose)."""
    import ml_dtypes
    xr = np.asarray(x_core, dtype=np.float32).reshape(T, 6, 128)
    return np.ascontiguousarray(
        xr.transpose(2, 1, 0).reshape(128, 6 * T)).astype(ml_dtypes.bfloat16)


def kernel(x, w_qkv, bias_table, w_proj, b_proj, rel_index):
    import time
    from concourse.bass_utils import run_bass_kernel_spmd

    x = np.asarray(x, dtype=np.float32)
    shared = host_prep(w_qkv, bias_table, w_proj, b_proj, rel_index)
    nc = get_nc()
    in_maps = []
    for c in range(N_CORES):
        m = {"xT": prep_x(x[c * B_PER:(c + 1) * B_PER])}
        m.update(shared)
        in_maps.append(m)
    # Transient NRT_EXEC_UNIT_UNRECOVERABLE failures have been observed on
    # this fabric; an identical retry passes, so guard the execution.
    last_exc = None
    for attempt in range(3):
        try:
            res = run_bass_kernel_spmd(nc, in_maps, core_ids=list(range(N_CORES)))
            break
        except Exception as e:
            last_exc = e
            time.sleep(2.0)
    else:
        raise last_exc
    out = np.concatenate(
        [res.results[c]["y"].reshape(B_PER, N_TOK, DIM) for c in range(N_CORES)],
        axis=0,
    )
    return out


# revision 84
# speedup vs baseline: 1.0001x; 1.0001x over previous
"""3D window attention (B=32, N=513, D=768, H=12) on 8 trn2 NeuronCores. v6

Data-parallel over batch (4 per core). bf16 datapath:
  host: x pre-transposed to xT [128, 6ci, T] (no on-device transpose phase);
        relative-position bias gathered and packed to fp8 x8-prescaled
        tables for PE DoubleRow injection; exact f32 exp(bias) for tails.
  B2:   q[g] = [128(2 heads), T] bf16, k[g] = [128, T] bf16, in SBUF;
        batch-outer so each per-batch xT DMA chunk unblocks 12 psum tiles.
  B:    v[b] = [128, 4 key-tiles + key-512 row, 12*(64+1)] bf16 with ones
        column (denominator ride-along); key-512 v rows at partition 0/64.
  C:    per (b,g): S = fp8-DR bias inject + K^T Q, exp(S*0.125) on ACT,
        AV [65,512] accumulation. Key 512 is a rank-1 path (partitions 0/64
        of one bank, hoisted to the iteration start). The tail query's ops
        are deferred one iteration; the U-normalize (recip -> partition
        broadcast -> mul) is queued first on DVE after the AV stop so the
        two psU banks recycle with minimum latency.
  D:    y = a @ Wp^T + b_proj, interleaved 2 passes/iteration into C
        (emitted before the normalize so its DVE add isn't queue-blocked),
        drained with psS-bank alternation at the end.
"""

import os
import numpy as np

PROBE = set(os.environ.get("KPROBE", "").split(","))

B, N_TOK, DIM, HEADS = 32, 513, 768, 12
HD = DIM // HEADS             # 64
N_CORES = 8
B_PER = B // N_CORES          # 4
T = B_PER * N_TOK             # 2052
G = HEADS // 2                # 6 head pairs
ES = 0.125                    # exp scale (= head_dim**-0.5)

_CACHE = {}


def _build_nc():
    import concourse.bacc as bacc
    import concourse.mybir as mybir
    import concourse.tile as tile
    from concourse.ap import AP

    F32 = mybir.dt.float32
    BF16 = mybir.dt.bfloat16
    FP8 = mybir.dt.float8e4
    AF = mybir.ActivationFunctionType
    DR = mybir.MatmulPerfMode.DoubleRow

    nc = bacc.Bacc("TRN2", target_bir_lowering=False, debug=False)

    xT_d = nc.dram_tensor("xT", [128, 6 * T], BF16, kind="ExternalInput")
    wqk_d = nc.dram_tensor("wqkT", [DIM, 2 * DIM], BF16, kind="ExternalInput")
    wv_d = nc.dram_tensor("wvT", [DIM, DIM], BF16, kind="ExternalInput")
    bt8_d = nc.dram_tensor("bt8", [G, 128, 8 * 512], FP8, kind="ExternalInput")
    bt4_d = nc.dram_tensor("bt4", [2, G * 512], FP8, kind="ExternalInput")
    id0_d = nc.dram_tensor("id0", [128, 256], FP8, kind="ExternalInput")
    e2_d = nc.dram_tensor("e2", [2, 256], FP8, kind="ExternalInput")
    ebt_d = nc.dram_tensor("ebt", [128, 10 * G], F32, kind="ExternalInput")
    wp_d = nc.dram_tensor("wpT", [DIM, DIM], BF16, kind="ExternalInput")
    y_d = nc.dram_tensor("y", [T, DIM], F32, kind="ExternalOutput")

    t_tiles = [(i * 128, min(128, T - i * 128)) for i in range((T + 127) // 128)]

    with tile.TileContext(nc) as tc:
        consts_cm = tc.tile_pool(name="consts", bufs=1)
        consts = consts_cm.__enter__()
        pers_cm = tc.tile_pool(name="pers", bufs=1)
        pers = pers_cm.__enter__()
        xTp_cm = tc.tile_pool(name="xTp", bufs=1)
        xTp = xTp_cm.__enter__()

        ebt = consts.tile([128, 10 * G], F32, tag="ebt", name="ebt")
        id0 = consts.tile([128, 256], FP8, tag="id0", name="id0")
        e2 = consts.tile([2, 256], FP8, tag="e2", name="e2")
        bt4S = consts.tile([2, G * 512], FP8, tag="bt4", name="bt4")

        qS = [pers.tile([128, T], BF16, tag=f"q{g}", name=f"q{g}")
              for g in range(G)]
        kS = [pers.tile([128, T], BF16, tag=f"k{g}", name=f"k{g}")
              for g in range(G)]
        btS = [pers.tile([128, 8 * 512], FP8, tag=f"bt{g}", name=f"bt{g}")
               for g in range(G)]
        v8 = [pers.tile([128, 5, HEADS * (HD + 1)], BF16, tag=f"v{b}",
                        name=f"v{b}") for b in range(B_PER)]
        wvp_cm = tc.tile_pool(name="wvp", bufs=1)
        wvp = wvp_cm.__enter__()
        wqp_cm = tc.tile_pool(name="wqp", bufs=1)
        wqp = wqp_cm.__enter__()
        wqk = wqp.tile([128, 6, 2 * DIM], BF16, tag="wqk", name="wqk")
        wv = wvp.tile([128, 6, DIM], BF16, tag="wv", name="wv")
        xT = xTp.tile([128, 6, T], BF16, tag="xT", name="xT")

        wqk_src = wqk_d.ap().rearrange("(c p) d -> p c d", p=128)
        wv_src = wv_d.ap().rearrange("(c p) d -> p c d", p=128)
        xT_src = xT_d.ap().rearrange("p (c t) -> p c t", t=T)

        # input DMAs, ordered so B2 unblocks earliest: wqk ci-chunks with
        # xT batch-chunks woven in (B2 consumes per-batch), then wv, bias.
        xT_bsrc = xT_d.ap().rearrange("p (c b n) -> p c b n", c=6, n=N_TOK)
        xT_bdst = xT.rearrange("p c (b n) -> p c b n", n=N_TOK)
        # fine-grained prelude: q-half of wqk-ci0, then xT ci-chunks of
        # batch 0 woven between wqk ci-chunks, so the first projection
        # matmuls start ~2.5us in and the PE p-state ramps without resets
        nc.sync.dma_start(wqk[:, 0, 0:DIM], wqk_src[:, 0, 0:DIM])
        nc.sync.dma_start(xT_bdst[:, 0, 0], xT_bsrc[:, 0, 0])
        nc.sync.dma_start(wqk[:, 0, DIM:2 * DIM], wqk_src[:, 0, DIM:2 * DIM])
        nc.sync.dma_start(xT_bdst[:, 1, 0], xT_bsrc[:, 1, 0])
        nc.sync.dma_start(wqk[:, 1], wqk_src[:, 1])
        for ci in range(2, 6):
            nc.sync.dma_start(xT_bdst[:, ci, 0], xT_bsrc[:, ci, 0])
            nc.sync.dma_start(wqk[:, ci], wqk_src[:, ci])
        for b in range(1, B_PER):
            nc.sync.dma_start(xT_bdst[:, :, b], xT_bsrc[:, :, b])
        for ci in range(6):
            nc.sync.dma_start(wv[:, ci], wv_src[:, ci])
        for g in range(G):
            nc.sync.dma_start(btS[g][:], bt8_d.ap()[g])
        nc.sync.dma_start(ebt[:], ebt_d.ap())
        nc.sync.dma_start(id0[:], id0_d.ap())
        nc.sync.dma_start(e2[:], e2_d.ap())
        nc.sync.dma_start(bt4S[:], bt4_d.ap())
        id0p = id0.rearrange("p (i c) -> p i c", c=128)
        e2p = e2.rearrange("p (i c) -> p i c", c=128)

        for b in range(B_PER):
            v3 = v8[b].rearrange("p m (h x) -> p m h x", x=HD + 1)
            nc.gpsimd.memset(v3[:, :, :, HD], 1.0)

        # ---------------- Phase B2: q / k in SBUF ----------------
        pB2_ps = tc.tile_pool(name="psB2", bufs=7, space="PSUM")
        pB2_pt = tc.tile_pool(name="psB2t", bufs=1, space="PSUM")
        with pB2_ps as psB2, pB2_pt as psB2t:
            cast_i = 0

            def cast(out, in_):
                nonlocal cast_i
                cast_i += 1
                if cast_i % 2 == 0:
                    nc.vector.tensor_copy(out, in_)
                else:
                    nc.scalar.copy(out, in_)

            # batch-outer so each xT batch-chunk DMA unblocks 12 full
            # psum tiles of projection work (q and k, all 6 groups)
            for b in range(B_PER):
                for g in range(G):
                    psq = psB2.tile([128, 512], F32, tag="psq", name="psq")
                    for ci in range(6):
                        nc.tensor.matmul(
                            psq[:], wqk[:, ci, g * 128:(g + 1) * 128],
                            xT[:, ci, b * N_TOK:b * N_TOK + 512],
                            start=(ci == 0), stop=(ci == 5))
                    cast(qS[g][:, b * N_TOK:b * N_TOK + 512], psq[:])
                    psk = psB2.tile([128, 512], F32, tag="psq", name="psk")
                    for ci in range(6):
                        nc.tensor.matmul(
                            psk[:], wqk[:, ci, DIM + g * 128:DIM + (g + 1) * 128],
                            xT[:, ci, b * N_TOK:b * N_TOK + 512],
                            start=(ci == 0), stop=(ci == 5))
                    cast(kS[g][:, b * N_TOK:b * N_TOK + 512], psk[:])
            for g in range(G):
                psqt = psB2t.tile([128, 4], F32, tag="psqt", name="psqt")
                for ci in range(6):
                    nc.tensor.matmul(
                        psqt[:], wqk[:, ci, g * 128:(g + 1) * 128],
                        xT.rearrange("p c (b n) -> p c b n", n=N_TOK)
                          [:, ci, :, 512],
                        start=(ci == 0), stop=(ci == 5))
                cast(qS[g][:, 0:T].rearrange(
                         "p (b w) -> p b w", w=N_TOK)[:, :, 512],
                     psqt[:])
                psk4 = psB2t.tile([128, 4], F32, tag="psqt", name="psk4")
                for ci in range(6):
                    nc.tensor.matmul(
                        psk4[:], wqk[:, ci, DIM + g * 128:DIM + (g + 1) * 128],
                        xT.rearrange("p c (b n) -> p c b n", n=N_TOK)
                          [:, ci, :, 512],
                        start=(ci == 0), stop=(ci == 5))
                cast(kS[g][:, 0:T].rearrange(
                         "p (b w) -> p b w", w=N_TOK)[:, :, 512],
                     psk4[:])

        wqp_cm.__exit__(None, None, None)

        # ---------------- Phase B: v (per-batch key-tile layout) ---------
        pB_ps = tc.tile_pool(name="psB", bufs=3, space="PSUM")
        pB_pt = tc.tile_pool(name="psBt", bufs=1, space="PSUM")
        with pB_ps as psB, pB_pt as psBt:
            for b in range(B_PER):
                v3 = v8[b].rearrange("p m (h x) -> p m h x", x=HD + 1)
                for mt in range(4):
                    psv = psB.tile([128, DIM], F32, tag="psv", name="psv")
                    lhs0 = b * N_TOK + mt * 128
                    for ci in range(6):
                        lhsT = xT[:, ci, lhs0:lhs0 + 128]
                        nc.tensor.matmul(psv[:, 0:512], lhsT,
                                         wv[:, ci, 0:512],
                                         start=(ci == 0), stop=(ci == 5))
                        nc.tensor.matmul(psv[:, 512:768], lhsT,
                                         wv[:, ci, 512:768],
                                         start=(ci == 0), stop=(ci == 5))
                    src = psv[:].rearrange("p (h d) -> p h d", d=HD)
                    if b % 2 == 0:
                        nc.vector.tensor_copy(v3[:, mt, :, 0:HD], src)
                    else:
                        nc.scalar.copy(v3[:, mt, :, 0:HD], src)
            psvt = psBt.tile([128, DIM], F32, tag="psvt", name="psvt")
            for ci in range(6):
                lhsT = xT.rearrange("p c (b n) -> p c b n", n=N_TOK)[:, ci, :, 512]
                nc.tensor.matmul(psvt[:B_PER, 0:512], lhsT, wv[:, ci, 0:512],
                                 start=(ci == 0), stop=(ci == 5))
                nc.tensor.matmul(psvt[:B_PER, 512:768], lhsT, wv[:, ci, 512:768],
                                 start=(ci == 0), stop=(ci == 5))
            vst = pers.tile([B_PER, DIM], BF16, tag="vst", name="vst")
            nc.vector.tensor_copy(vst[:], psvt[:B_PER, :])
            for b in range(B_PER):
                v3 = v8[b].rearrange("p m (h x) -> p m h x", x=HD + 1)
                vsrc = vst.rearrange("p (h d) -> p h d", d=HD)
                # key-512 v rows: even heads (h2=0) at partition 0, odd heads
                # (h2=1) at partition 64, matching the pmb4/ptail4 row homes
                nc.sync.dma_start(v3[0:1, 4, 0:12:2, 0:HD],
                                  vsrc[b:b + 1, 0:12:2])
                nc.sync.dma_start(v3[HD:HD + 1, 4, 1:12:2, 0:HD],
                                  vsrc[b:b + 1, 1:12:2])

        wvp_cm.__exit__(None, None, None)
        xTp_cm.__exit__(None, None, None)

        # ---------------- Phase C + interleaved D ----------------
        aTp_cm = tc.tile_pool(name="aTp", bufs=1)
        aTp = aTp_cm.__enter__()
        aT = [aTp.tile([128, T], BF16, tag=f"aT{g}", name=f"aT{g}")
              for g in range(G)]
        wpal = aTp.tile([128, 6, DIM], BF16, tag="wpal", name="wpal")
        wp_src = wp_d.ap().rearrange("(c p) d -> p c d", p=128)
        for ci in range(6):
            nc.sync.dma_start(wpal[:, ci], wp_src[:, ci])
        pC_pm = tc.tile_pool(name="pmp", bufs=4)
        pC_pt = tc.tile_pool(name="ptp", bufs=2)
        pC_sm = tc.tile_pool(name="smallp", bufs=3)
        pD3 = tc.tile_pool(name="ytp", bufs=3)
        pS_ps = tc.tile_pool(name="psS", bufs=2, space="PSUM")
        pU_ps = tc.tile_pool(name="psU", bufs=2, space="PSUM")
        pT_ps = tc.tile_pool(name="psT", bufs=1, space="PSUM")
        pD_ps = tc.tile_pool(name="psD", bufs=1, space="PSUM")
        d_done = [0]
        d_half = [0]
        d_yt = [None]
        d_avail = [0]
        with pC_pm as pmp, \
             pC_pt as ptp, pC_sm as smallp, pD3 as ytp, \
             pS_ps as psS, pU_ps as psU, pT_ps as psT, pD_ps as psD:

            def emit_d_pass(alt=False):
                # one 6-matmul pass (half an output tile) per call; the
                # half-0 call sits right before the normalize block so its
                # yt-add beats the norm muls onto the DVE queue.
                # alt=True (drain) alternates banks via the idle psS pool.
                if d_done[0] >= len(t_tiles):
                    return False
                t0, ts = t_tiles[d_done[0]]
                if t0 + ts > d_avail[0]:
                    return False
                half = d_half[0]
                (c0, cw) = ((0, 512), (512, 256))[half]
                if alt and half == 1:
                    psy = psS.tile([128, 1024], F32, tag="S",
                                   name="psyS")[:, 0:512]
                else:
                    psy = psD.tile([128, 512], F32, tag="psy", name="psy")
                for j in range(6):
                    nc.tensor.matmul(psy[:ts, 0:cw], aT[j][:, t0:t0 + ts],
                                     wpal[:, j, c0:c0 + cw],
                                     start=(j == 0), stop=(j == 5))
                if half == 0:
                    yt = ytp.tile([128, DIM], F32, tag="yt", name="yt")
                    d_yt[0] = yt
                    if alt:
                        nc.scalar.copy(yt[:ts, 0:512], psy[:ts, 0:512])
                    else:
                        nc.vector.tensor_copy(yt[:ts, 0:512], psy[:ts, 0:512])
                    d_half[0] = 1
                else:
                    yt = d_yt[0]
                    if alt:
                        nc.scalar.copy(yt[:ts, 512:768], psy[:ts, 0:256])
                    else:
                        nc.vector.tensor_copy(yt[:ts, 512:768],
                                              psy[:ts, 0:256])
                    nc.sync.dma_start(y_d.ap()[t0:t0 + ts, :], yt[:ts, :])
                    d_half[0] = 0
                    d_done[0] += 1
                return True

            def emit_tail_av(st):
                # AV for the tail query of iteration st (ptail ready by now)
                b, g = st["b"], st["g"]
                for h2 in range(2):
                    h = 2 * g + h2
                    for mt in range(4):
                        nc.tensor.matmul(
                            st["stail"][0:HD + 1, 10 + h2:11 + h2],
                            v8[b][:, mt, h * 65:h * 65 + 65],
                            st["ptail"][:, 2 * mt + h2:2 * mt + h2 + 1],
                            start=(mt == 0), stop=False)
                    nc.tensor.matmul(
                        st["stail"][0:HD + 1, 10 + h2:11 + h2],
                        v8[b][h2 * HD:h2 * HD + 1, 4, h * 65:h * 65 + 65],
                        st["ptail"][h2 * HD:h2 * HD + 1, 8 + h2:9 + h2],
                        start=False, stop=True)

            def emit_norm_tail(st):
                b, g = st["b"], st["g"]
                rnt = smallp.tile([1, 2], F32, tag="rnt", name="rnt")
                nc.vector.reciprocal(rnt[:], st["stail"][HD:HD + 1, 10:12])
                for h2 in range(2):
                    bct = smallp.tile([HD, 1], F32, tag="bct", name="bct")
                    nc.gpsimd.partition_broadcast(bct[:], rnt[:, h2:h2 + 1])
                    nc.vector.tensor_mul(
                        aT[g][h2 * HD:h2 * HD + HD,
                              b * N_TOK + 512:b * N_TOK + 513],
                        st["stail"][0:HD, 10 + h2:11 + h2], bct[:])

            prev = [None]

            for b in range(B_PER):
                for g in range(G):
                    q, k = qS[g], kS[g]
                    q0 = b * N_TOK
                    btp = btS[g][:]
                    bt_pitch = btp.ap[0][0]

                    def s_main(mt):
                        # fp8 DoubleRow bias inject (half PE cost), then K^T Q
                        sm = psS.tile([128, 1024], F32, tag="S", name="S")
                        for h2 in range(2):
                            nc.tensor.matmul(
                                sm[:, h2 * 512:h2 * 512 + 512], id0p[:],
                                AP(btp.tensor, (h2 * 4 + mt) * 512,
                                   [[bt_pitch, 128], [0, 2], [1, 512]]),
                                start=True, stop=False, perf_mode=DR)
                            nc.tensor.matmul(
                                sm[:, h2 * 512:h2 * 512 + 512],
                                k[h2 * HD:h2 * HD + HD,
                                  q0 + mt * 128:q0 + mt * 128 + 128],
                                q[h2 * HD:h2 * HD + HD, q0:q0 + 512],
                                start=False, stop=True)
                        return sm

                    def exp_main(sm):
                        pm = pmp.tile([128, 1024], BF16, tag="pm", name="pm")
                        nc.scalar.activation(pm[:], sm[:], AF.Exp, bias=0.0,
                                             scale=ES)
                        return pm

                    def emit_av(mt, pmb, stop=False):
                        for h2 in range(2):
                            h = 2 * g + h2
                            nc.tensor.matmul(
                                U2[h2][:],
                                v8[b][:, mt, h * 65:h * 65 + 65],
                                pmb[:, h2 * 512:h2 * 512 + 512],
                                start=(mt == 0), stop=stop)

                    if g % 2 == 0:
                        emit_d_pass()
                    U2 = [psU.tile([HD + 1, 512], F32, tag="U", name="U")
                          for _ in range(2)]
                    # key-512 rank-1 row first: exp4 fills the ACT engine's
                    # iteration-boundary idle instead of delaying exp2/exp3
                    if "noexp4" not in PROBE:
                        sm4 = psS.tile([128, 1024], F32, tag="S", name="S4")
                        bt4p = bt4S[:]
                        nc.tensor.matmul(
                            sm4[:, 0:512], e2p[:],
                            AP(bt4p.tensor, g * 512,
                               [[bt4p.ap[0][0], 2], [0, 2], [1, 512]]),
                            start=True, stop=True, perf_mode=DR)
                        for h2 in range(2):
                            nc.tensor.matmul(
                                sm4[h2 * HD:h2 * HD + 1, 0:512],
                                k[h2 * HD:h2 * HD + HD, q0 + 512:q0 + 513],
                                q[h2 * HD:h2 * HD + HD, q0:q0 + 512],
                                start=False, stop=True)
                        pm4 = pmp.tile([128, 1024], BF16, tag="pm", name="pm4")
                        nc.scalar.activation(pm4[:, 0:512], sm4[:, 0:512],
                                             AF.Exp, bias=0.0, scale=ES)
                    sm0 = s_main(0)
                    pm0 = exp_main(sm0)
                    sm1 = s_main(1)
                    if prev[0] is not None:
                        emit_tail_av(prev[0])
                        if g == 0 and b > 0:
                            d_avail[0] = b * N_TOK
                    pm1 = exp_main(sm1)
                    sm2 = s_main(2)
                    if prev[0] is not None:
                        emit_norm_tail(prev[0])
                    pm2 = exp_main(sm2)
                    sm3 = s_main(3)
                    emit_av(0, pm0)
                    emit_av(1, pm1)
                    # key-512 rank-1 AV early (AV3 is the group stop)
                    if "noexp4" not in PROBE:
                        for h2 in range(2):
                            h = 2 * g + h2
                            nc.tensor.matmul(
                                U2[h2][:],
                                v8[b][h2 * HD:h2 * HD + 1, 4,
                                      h * 65:h * 65 + 65],
                                pm4[h2 * HD:h2 * HD + 1, 0:512],
                                start=False, stop=False)
                    pm3 = exp_main(sm3)
                    # tail-query logits (consumed next iteration)
                    if "notailq" not in PROBE:
                        stail = psT.tile([128, 12], F32, tag="st", name="st")
                        for h2 in range(2):
                            for mt in range(4):
                                nc.tensor.matmul(
                                    stail[:, 2 * mt + h2:2 * mt + h2 + 1],
                                    k[h2 * HD:h2 * HD + HD,
                                      q0 + mt * 128:q0 + mt * 128 + 128],
                                    q[h2 * HD:h2 * HD + HD, q0 + 512:q0 + 513],
                                    start=True, stop=True)
                            nc.tensor.matmul(
                                stail[h2 * HD:h2 * HD + 1, 8 + h2:9 + h2],
                                k[h2 * HD:h2 * HD + HD, q0 + 512:q0 + 513],
                                q[h2 * HD:h2 * HD + HD, q0 + 512:q0 + 513],
                                start=True, stop=True)
                        praw = smallp.tile([128, 10], F32, tag="praw",
                                           name="praw")
                        nc.scalar.activation(praw[:], stail[:, 0:10], AF.Exp,
                                             bias=0.0, scale=ES)
                    emit_av(2, pm2)
                    emit_av(3, pm3, stop=True)
                    emit_d_pass()
                    # normalize main block FIRST on the DVE queue after the
                    # U-group stop, so the psU buffers free with minimum
                    # latency (next iteration's AV(0) reuses them).
                    rns = []
                    for h2 in range(2):
                        if "norecip" in PROBE:
                            continue
                        rn = smallp.tile([1, 512], F32, tag="rn", name="rn")
                        nc.vector.reciprocal(rn[:], U2[h2][HD:HD + 1, :])
                        rns.append(rn)
                    for h2 in range(2):
                        if "norecip" in PROBE:
                            nc.vector.tensor_copy(
                                aT[g][h2 * HD:h2 * HD + HD, q0:q0 + 512],
                                U2[h2][0:HD, :])
                            continue
                        bc = smallp.tile([HD, 512], F32, tag="bc", name="bc")
                        nc.gpsimd.partition_broadcast(bc[:], rns[h2][:])
                        nc.vector.tensor_mul(
                            aT[g][h2 * HD:h2 * HD + HD, q0:q0 + 512],
                            U2[h2][0:HD, :], bc[:])
                    if "notailq" not in PROBE:
                        ptail = ptp.tile([128, 10], BF16, tag="pt", name="pt")
                        nc.vector.tensor_mul(ptail[:], praw[:],
                                             ebt[:, g * 10:g * 10 + 10])
                        prev[0] = {"b": b, "g": g, "stail": stail,
                                   "ptail": ptail}
                    elif g == 0 and b > 0:
                        d_avail[0] = b * N_TOK

            if prev[0] is not None:
                emit_tail_av(prev[0])
                emit_norm_tail(prev[0])
            d_avail[0] = T
            while emit_d_pass(alt=True):
                pass

        aTp_cm.__exit__(None, None, None)
        pers_cm.__exit__(None, None, None)
        consts_cm.__exit__(None, None, None)

    nc.compile()
    return nc


def get_nc():
    if "nc" not in _CACHE:
        _CACHE["nc"] = _build_nc()
    return _CACHE["nc"]


def host_prep(w_qkv, bias_table, w_proj, b_proj, rel_index):
    """Host-side packing shared by all cores."""
    import ml_dtypes
    BF = ml_dtypes.bfloat16
    w = np.asarray(w_qkv, dtype=np.float32)
    wqkT = np.ascontiguousarray(w[0:2 * DIM].T).astype(BF)     # [c, 1536]
    wvT = np.ascontiguousarray(w[2 * DIM:3 * DIM].T).astype(BF)

    E4 = ml_dtypes.float8_e4m3
    BPK = 8.0                                   # bias prepack = 1/ES
    tbl = np.asarray(bias_table, dtype=np.float32)
    gat = tbl[np.asarray(rel_index)]            # [n(query), m(key), h]
    Bm = gat.transpose(2, 1, 0)                 # [h, m(key), n(query)]
    EBm = np.exp(Bm)                            # exp(bias) for the tail path

    # mt 2/3: fp8 pre-scaled (x8) bias tables, injected into S on the PE;
    # mt 0/1: exact bf16 exp(bias) factors, applied on the DVE after exp
    bt8 = np.zeros((G, 128, 8, 512), dtype=np.float32)
    for g in range(G):
        for h2 in range(2):
            for mt in range(4):
                bt8[g, :, h2 * 4 + mt, :] = \
                    BPK * Bm[2 * g + h2, mt * 128:mt * 128 + 128, 0:512]
    bt8 = np.clip(bt8, -240, 240).reshape(G, 128, 4096).astype(E4)

    # key-512 row biases: bt4 rows 0 / 2 pair with e2 one-hots at cols 0 / 64
    bt4 = np.zeros((2, G * 512), dtype=np.float32)
    for g in range(G):
        for h2 in range(2):
            bt4[h2, g * 512:g * 512 + 512] = \
                BPK * Bm[2 * g + h2, 512, 0:512]
    bt4 = np.clip(bt4, -240, 240).astype(E4)

    id0 = np.zeros((128, 256), dtype=np.float32)
    id0[:, 0:128] = np.eye(128, dtype=np.float32)
    id0 = id0.astype(E4)
    e2 = np.zeros((2, 256), dtype=np.float32)
    e2[0, 0] = 1.0      # bt4 row 0 -> sm4 partition 0   (h2=0)
    e2[1, HD] = 1.0     # bt4 row 1 -> sm4 partition 64  (h2=1)
    e2 = e2.astype(E4)

    ebt = np.zeros((128, 10 * G), dtype=np.float32)
    for g in range(G):
        for mt in range(4):
            for h2 in range(2):
                ebt[:, g * 10 + 2 * mt + h2] = \
                    EBm[2 * g + h2, mt * 128:mt * 128 + 128, 512]
        for h2 in range(2):
            # cols 8/9: key-512 factor at row h2*HD (the partition home of
            # the rank-1 path); other rows stay 0 and mask stale exp lanes
            ebt[h2 * HD, g * 10 + 8 + h2] = EBm[2 * g + h2, 512, 512]

    wpT = np.ascontiguousarray(np.asarray(w_proj, dtype=np.float32).T).astype(BF)
    return {"wqkT": wqkT, "wvT": wvT, "bt8": bt8, "bt4": bt4,
            "id0": id0, "e2": e2, "ebt": ebt, "wpT": wpT}


def prep_x(x_core):
    """[B_PER, N_TOK, DIM] f32 -> xT [128, 6*T] bf16 (host transpose)."""
    import ml_dtypes
    xr = np.asarray(x_core, dtype=np.float32).reshape(T, 6, 128)
    return np.ascontiguousarray(
        xr.transpose(2, 1, 0).reshape(128, 6 * T)).astype(ml_dtypes.bfloat16)


def kernel(x, w_qkv, bias_table, w_proj, b_proj, rel_index):
    import time
    from concourse.bass_utils import run_bass_kernel_spmd

    x = np.asarray(x, dtype=np.float32)
    shared = host_prep(w_qkv, bias_table, w_proj, b_proj, rel_index)
    nc = get_nc()
    in_maps = []
    for c in range(N_CORES):
        m = {"xT": prep_x(x[c * B_PER:(c + 1) * B_PER])}
        m.update(shared)
        in_maps.append(m)
    # Transient NRT_EXEC_UNIT_UNRECOVERABLE failures have been observed on
    # this fabric; an identical retry passes, so guard the execution.
    last_exc = None
    for attempt in range(3):
        try:
            res = run_bass_kernel_spmd(nc, in_maps, core_ids=list(range(N_CORES)))
            break
        except Exception as e:
            last_exc = e
            time.sleep(2.0)
    else:
        raise last_exc
    out = np.concatenate(
        [res.results[c]["y"].reshape(B_PER, N_TOK, DIM) for c in range(N_CORES)],
        axis=0,
    )
    # b_proj is added host-side so the device yt ops are pure psum->sbuf
    # copies (the drain's run on the otherwise-idle ACT engine)
    out += np.asarray(b_proj, dtype=np.float32)[None, None, :]
    return out
